# revision 37
# baseline (speedup 1.0000x reference)
"""Trainium2 Bass kernel for nn_MaxMinAgg.

Computes, for full inputs m [1024, 256] f32 and weight [256, 512] f32:
    z[b, j]  = max_k min(m[b, k], weight[k, j])          (tropical max-min matmul)
    out[b,o] = max_a z[b, 4*o + a]                       (max-pool over AGG=4 groups)

Identity: max_a min(x, w_a) = min(x, max_a w_a): the AGG max-pool folds into the
weight (wmax[k, o] = max_a weight[k, 4o+a]), so
    out[b, o] = max_k min(m[b, k], wmax[k, o])

Distribution: data-parallel over batch across 8 NeuronCores (128 rows each);
weight replicated.

Per-core algorithm (final — bf16, low-replication layout, max-tree, short DMA
chains, PE-built early weight columns):
  - Chunk 1's last EPC=6 o'-columns of w_rep are built ON-CHIP: host-fed 0/1
    selection matrices (wmask0) are matmul'd against wmaxT on the idle PE and
    CAST-copied PSUM->SBUF by the DVE, so the DVE starts hot min/tree work
    ~8us before the first replicated w_rep DMA lands (fills the startup idle
    window that the DRAM-roundtrip latency otherwise leaves).
  - All compute in bf16: min/max are exact selections, so the only error is the
    initial f32->bf16 rounding (<= 2^-9 relative, far under the 2e-2 gate), and
    bf16 unlocks the DVE's 2x packed mode for tensor_tensor (tensor_reduce is
    always 1x, hence the halving max-tree below).
  - Partition layout p = og*16 + bg (OG=8 output groups x BG=16 batch groups).
    Partition p owns the full k-reduction for an 8b' x 16o' output tile, so
    k (=256) lives on the free axis and there is NO cross-partition combine.
  - m path: ONE replicated broadcast read straight from the input (1MB f32,
    8KB-contiguous runs, two b'-half chunks solo on the sync queue), then two
    ACT converts produce m16rep [p, b', k] bf16 — no DRAM scratch hop.
  - w path: 512KB load split across both HW queues -> per-half DVE fold
    (output o-permuted CHUNK-MAJOR) + PE transpose + ACT copy -> wmaxT bf16 ->
    per-chunk 32KB DRAM writes -> per-chunk broadcast reads give w_rep
    [p, o', k] (transpose outputs must start at PSUM partition 0, so direct
    placement at partition offsets is impossible; the chunk-major permutation
    lets each wT chunk ship as soon as it exists).
  - DMA scheduling (from packet-level traces): contiguous streams run at
    200-340GB/s per HW queue, but REPLICATED broadcast reads cap at ~130GB/s
    and collapse when several run concurrently, and every hop costs ~2us of
    issue->data latency. So the m broadcast owns sync, w_rep owns scalar, and
    chunk 0 of each is sub-split so the first min gates on partial data.
  - Hot loop per o'-half: bf16 tensor_tensor min (m16rep broadcast over o',
    w_rep broadcast over b'; chunk 0's min runs as a 2x2 (b'-half x o'-half)
    sub-grid so it starts as soon as the first quarter of its inputs lands),
    a halving tensor_tensor max tree over k (256->...->16, all 2x, contiguous
    halves so packing holds), then one 1x tensor_reduce of the last 16
    straight to f32 and a scatter-DMA to the natural [b, o] layout.
"""

import sys

import numpy as np

if "/opt/trn_rl_repo" not in sys.path:
    sys.path.insert(0, "/opt/trn_rl_repo")

B, IN_F, OUT_F, AGG = 1024, 256, 128, 4
N_CORES = 8
B_SH = B // N_CORES  # 128

OG, BG = 8, 16  # partition p = og*BG + bg
BP = B_SH // BG  # 8 batch rows per partition
OPP = OUT_F // OG  # 16 output cols per partition
NT = 2  # o'-halves (chunks of the hot loop)
OC = OPP // NT  # 8 output cols per chunk

_CACHE = {}


EPC = 6  # chunk-1 o'-columns built on-chip via PE selection-matmuls


def _identity_np():
    import ml_dtypes

    return np.eye(128, dtype=ml_dtypes.bfloat16)


def _wmask_np():
    """Selection matrices for the PE-built w_rep columns: wmask[q, j, p] = 1
    iff wmaxT (chunk-major) row q is the one partition p needs for chunk 1's
    column oc = OC - EPC + j, i.e. q = 64 + (p//BG)*OC + (OC - EPC + j)."""
    import ml_dtypes

    q = np.arange(128)[:, None, None]
    p = np.arange(128)[None, None, :]
    j = np.arange(EPC)[None, :, None]
    sel = 64 + (p // BG) * OC + (OC - EPC + j)
    return (q == sel).astype(ml_dtypes.bfloat16)


def emit_core_program(tc, o_d, m_d, w_d, i_d, wm_d):
    """Emit the per-core Tile program.

    o_d: DRAM out [B_SH, OUT_F] f32, m_d: DRAM in [B_SH, IN_F] f32,
    w_d: DRAM in [IN_F, OUT_F*AGG] f32, i_d: DRAM in [128, 128] bf16 identity,
    wm_d: DRAM in [128, EPC, 128] bf16 selection masks (see _wmask_np).
    """
    from contextlib import ExitStack

    import concourse.bass as bass
    from concourse import mybir

    nc = tc.nc
    f32 = mybir.dt.float32
    bf16 = mybir.dt.bfloat16
    AX = mybir.AxisListType
    OP = mybir.AluOpType

    with ExitStack() as ctx:
        const = ctx.enter_context(tc.tile_pool(name="const", bufs=1))
        mintp = ctx.enter_context(tc.tile_pool(name="mintp", bufs=2))
        treep = ctx.enter_context(tc.tile_pool(name="treep", bufs=2))
        ps_tr = ctx.enter_context(tc.tile_pool(name="ps_tr", bufs=2, space="PSUM"))

        # --- queue heads. sync: the m natural load (tiny, unblocks the whole
        # m chain) then its w k-half then ident; scalar: the other w k-half.
        m32n = const.tile([128, IN_F], f32)
        nc.sync.dma_start(out=m32n, in_=m_d)
        w_sb = const.tile([128, 2, OUT_F * AGG], f32)
        wv = w_d.rearrange("(h p) j -> p h j", p=128)
        nc.scalar.dma_start(out=w_sb[:, 0, :], in_=wv[:, 0, :])
        nc.sync.dma_start(out=w_sb[:, 1, :], in_=wv[:, 1, :])
        ident = const.tile([128, 128], bf16)
        nc.sync.dma_start(out=ident, in_=i_d)
        wmask = const.tile([128, EPC, 128], bf16)
        nc.scalar.dma_start(out=wmask, in_=wm_d)

        # m -> bf16 (one tiny, early ACT op) -> DRAM: the broadcast then moves
        # 512KB of already-converted data, and the ACT stream stays clear of
        # the w-chain DMA issues during the hot window.
        m16n = const.tile([128, IN_F], bf16)
        nc.scalar.copy(out=m16n, in_=m32n)
        m16_d = nc.dram_tensor("m16_scratch", [B_SH, IN_F], bf16, kind="Internal").ap()
        nc.sync.dma_start(out=m16_d, in_=m16n)

        # m16rep[p = og*BG+bg, b', k] = m16[bg*BP + b', k], two b'-half chunks
        # solo on the sync queue (chunk 0 unblocks the first min early).
        m16rep = const.tile([128, BP, IN_F], bf16)
        HB = BP * IN_F // 2
        for c in range(2):
            src = bass.AP(
                tensor=m16_d.tensor,
                offset=m16_d.offset + c * HB,
                ap=[[0, OG], [BP * IN_F, BG], [1, HB]],
            )
            half = slice(c * (BP // 2), (c + 1) * (BP // 2))
            nc.sync.dma_start(out=m16rep[:, half, :], in_=src)

        # --- weight fold + transpose, pipelined per k-half h (h=1 lands on
        # the sync queue first). The fold output is o-permuted CHUNK-MAJOR:
        # wmax16[:, h, q] with q = t*64 + og*8 + o'  holds  o = og*16+t*8+o',
        # so wmaxT's partitions [t*64, t*64+64) are exactly chunk t's rows.
        wmax16 = const.tile([128, 2, OUT_F], bf16)
        wmaxT16 = const.tile([128, 2, 128], bf16)
        for h in (1, 0):
            # j = (og*16 + t*8 + o')*4 + a  ==  "(og t o a)" order.
            nc.vector.tensor_reduce(
                out=wmax16[:, h, :].rearrange("p (t og o) -> p t og o", t=NT, og=OG),
                in_=w_sb[:, h, :].rearrange(
                    "p (og t o a) -> p t og o a", og=OG, t=NT, a=AGG
                ),
                axis=AX.X,
                op=OP.max,
            )
            pt = ps_tr.tile([128, 128], bf16, tag="ptr")
            nc.tensor.transpose(pt, wmax16[:, h, :], ident)
            nc.scalar.copy(out=wmaxT16[:, h, :], in_=pt)

        # Per-chunk wT write + replicated read (solo on the scalar queue):
        # w_rep[t][p = og*BG+bg, o', k] = wmaxT row t*64 + og*8 + o'.
        wT_d = nc.dram_tensor("wT_scratch", [OUT_F, IN_F], bf16, kind="Internal").ap()
        wT_v = wT_d.rearrange("(t r) k -> t r k", t=NT)
        # Chunk 1's last EPC columns skip the roundtrip entirely: PE selection
        # matmuls place them in PSUM straight from wmaxT16 (so the DVE can
        # start hot work ~8us before the first w_rep DMA lands).
        wreps = []
        for t in range(NT):
            nc.scalar.dma_start(
                out=wT_v[t], in_=wmaxT16[t * 64 : (t + 1) * 64, :, :]
            )
            noc = OC if t == 0 else OC - EPC
            wr = const.tile([128, noc, IN_F], bf16, name=f"wrep{t}")
            nsub = 2 if t == 0 else 1
            oc_s = noc // nsub
            for s in range(nsub):
                src = bass.AP(
                    tensor=wT_d.tensor,
                    offset=wT_d.offset + (t * 64 + s * oc_s) * IN_F,
                    ap=[[OC * IN_F, OG], [0, BG], [1, oc_s * IN_F]],
                )
                nc.scalar.dma_start(out=wr[:, s * oc_s : (s + 1) * oc_s, :], in_=src)
            wreps.append(wr)

        wmaxT_flat = wmaxT16.rearrange("p h k -> p (h k)")
        we = const.tile([128, EPC, IN_F], bf16, name="we")
        pses = []
        for j in range(EPC):
            pse = ps_tr.tile([128, IN_F], f32, tag=f"pse{j}", bufs=1)
            nc.tensor.matmul(out=pse, lhsT=wmask[:, j, :], rhs=wmaxT_flat)
            pses.append(pse)

        m16v = m16rep.rearrange("p b k -> p b () k")
        part = treep.tile([128, BP, OUT_F // OG], f32, tag="part", bufs=1)

        # --- early hot block on the PE-built columns (o' = 16-EPC .. 16) ----
        for j in range(EPC):
            nc.vector.tensor_copy(we[:, j, :], pses[j])
        mint_e = mintp.tile([128, BP, EPC, IN_F], bf16, tag="mint_e", bufs=1)
        for sb in range(2):  # b'-halves, gated on the two m16rep chunks
            bs = BP // 2
            nc.vector.tensor_tensor(
                out=mint_e[:, sb * bs : (sb + 1) * bs, :, :],
                in0=m16v[:, sb * bs : (sb + 1) * bs, :, :].broadcast_to(
                    (128, bs, EPC, IN_F)
                ),
                in1=we.rearrange("p o k -> p () o k").broadcast_to(
                    (128, bs, EPC, IN_F)
                ),
                op=OP.min,
            )
        cur, n = mint_e, IN_F
        while n > 16:
            h = n // 2
            nxt = treep.tile([128, BP, EPC, h], bf16, tag=f"etree{h}", bufs=1)
            nc.vector.tensor_tensor(
                out=nxt, in0=cur[:, :, :, :h], in1=cur[:, :, :, h:], op=OP.max
            )
            cur, n = nxt, h
        nc.vector.tensor_reduce(
            out=part[:, :, OPP - EPC :], in_=cur, axis=AX.X, op=OP.max
        )

        # --- hot loop: per o'-chunk, bf16 min + halving max-tree over k -----
        for t in range(NT):
            noc = OC if t == 0 else OC - EPC
            mint = mintp.tile([128, BP, OC, IN_F], bf16, tag="mint")
            # Chunk 0's min runs as a (b'-half x o'-half) sub-grid gated on
            # the m/w sub-chunks; chunk 1 is one op.
            grid = (2, 1)[t]
            bs, os_ = BP // grid, noc // grid
            for sb in range(grid):
                for so in range(grid):
                    nc.vector.tensor_tensor(
                        out=mint[
                            :, sb * bs : (sb + 1) * bs, so * os_ : (so + 1) * os_, :
                        ],
                        in0=m16v[:, sb * bs : (sb + 1) * bs, :, :].broadcast_to(
                            (128, bs, os_, IN_F)
                        ),
                        in1=wreps[t][:, so * os_ : (so + 1) * os_, :]
                        .rearrange("p o k -> p () o k")
                        .broadcast_to((128, bs, os_, IN_F)),
                        op=OP.min,
                    )
            cur, n = mint[:, :, :noc, :], IN_F
            while n > 16:
                h = n // 2
                nxt = treep.tile([128, BP, OC, h], bf16, tag=f"tree{h}")
                nc.vector.tensor_tensor(
                    out=nxt[:, :, :noc, :],
                    in0=cur[:, :, :, :h],
                    in1=cur[:, :, :, h:],
                    op=OP.max,
                )
                cur, n = nxt[:, :, :noc, :], h
            nc.vector.tensor_reduce(
                out=part[:, :, t * OC : t * OC + noc], in_=cur, axis=AX.X, op=OP.max
            )

        # Both chunks' o' are adjacent in `part`, so one scatter with 64B
        # contiguous runs writes the natural [b, o] layout.
        dst = bass.AP(
            tensor=o_d.tensor,
            offset=o_d.offset,
            ap=[[OPP, OG], [BP * OUT_F, BG], [OUT_F, BP], [1, OPP]],
        )
        nc.sync.dma_start(out=dst, in_=part)


def _build():
    if "nc" in _CACHE:
        return _CACHE["nc"]
    import concourse.bacc as bacc
    import concourse.tile as tile
    from concourse import mybir

    f32 = mybir.dt.float32
    bf16 = mybir.dt.bfloat16
    nc = bacc.Bacc(
        "TRN2",
        target_bir_lowering=False,
        debug=False,
        enable_asserts=True,
        num_devices=N_CORES,
    )
    m_d = nc.dram_tensor("m0", [B_SH, IN_F], f32, kind="ExternalInput").ap()
    w_d = nc.dram_tensor("w0", [IN_F, OUT_F * AGG], f32, kind="ExternalInput").ap()
    i_d = nc.dram_tensor("ident0", [128, 128], bf16, kind="ExternalInput").ap()
    wm_d = nc.dram_tensor("wmask0", [128, EPC, 128], bf16, kind="ExternalInput").ap()
    o_d = nc.dram_tensor("out0", [B_SH, OUT_F], f32, kind="ExternalOutput").ap()
    with tile.TileContext(nc) as tc:
        emit_core_program(tc, o_d, m_d, w_d, i_d, wm_d)
    nc.compile()
    _CACHE["nc"] = nc
    return nc


def run(m, weight, trace=False, **spmd_kwargs):
    """Run on 8 NeuronCores; returns (full_output, BassKernelResults)."""
    from concourse.bass_utils import run_bass_kernel_spmd

    nc = _build()
    m = np.ascontiguousarray(np.asarray(m, dtype=np.float32))
    weight = np.ascontiguousarray(np.asarray(weight, dtype=np.float32))
    assert m.shape == (B, IN_F) and weight.shape == (IN_F, OUT_F * AGG)
    ident = _identity_np()
    wmask = _wmask_np()
    in_maps = [
        {
            "m0": m[i * B_SH : (i + 1) * B_SH],
            "w0": weight,
            "ident0": ident,
            "wmask0": wmask,
        }
        for i in range(N_CORES)
    ]
    res = run_bass_kernel_spmd(
        nc, in_maps, core_ids=list(range(N_CORES)), trace=trace, **spmd_kwargs
    )
    out = np.concatenate([res.results[i]["out0"] for i in range(N_CORES)], axis=0)
    return out, res


def kernel(m, weight, agg_features=AGG, **_ignored):
    assert int(agg_features) == AGG
    out, _ = run(m, weight, trace=False)
    return out.astype(np.float32)


# revision 38
# speedup vs baseline: 1.1529x; 1.1529x over previous
"""Trainium2 Bass kernel for nn_MaxMinAgg.

Computes, for full inputs m [1024, 256] f32 and weight [256, 512] f32:
    z[b, j]  = max_k min(m[b, k], weight[k, j])          (tropical max-min matmul)
    out[b,o] = max_a z[b, 4*o + a]                       (max-pool over AGG=4 groups)

Identity: max_a min(x, w_a) = min(x, max_a w_a): the AGG max-pool folds into the
weight (wmax[k, o] = max_a weight[k, 4o+a]), so
    out[b, o] = max_k min(m[b, k], wmax[k, o])

Distribution: data-parallel over batch across 8 NeuronCores (128 rows each);
weight replicated.

Per-core algorithm (final — bf16, low-replication layout, max-tree, short DMA
chains, PE-built early weight columns):
  - Chunk 1's last EPC=6 o'-columns of w_rep are built ON-CHIP: host-fed 0/1
    selection matrices (wmask0) are matmul'd against wmaxT on the idle PE and
    CAST-copied PSUM->SBUF by the DVE, so the DVE starts hot min/tree work
    ~8us before the first replicated w_rep DMA lands (fills the startup idle
    window that the DRAM-roundtrip latency otherwise leaves).
  - All compute in bf16: min/max are exact selections, so the only error is the
    initial f32->bf16 rounding (<= 2^-9 relative, far under the 2e-2 gate), and
    bf16 unlocks the DVE's 2x packed mode for tensor_tensor (tensor_reduce is
    always 1x, hence the halving max-tree below).
  - Partition layout p = og*16 + bg (OG=8 output groups x BG=16 batch groups).
    Partition p owns the full k-reduction for an 8b' x 16o' output tile, so
    k (=256) lives on the free axis and there is NO cross-partition combine.
  - m path: ONE replicated broadcast read straight from the input (1MB f32,
    8KB-contiguous runs, two b'-half chunks solo on the sync queue), then two
    ACT converts produce m16rep [p, b', k] bf16 — no DRAM scratch hop.
  - w path: 512KB load split across both HW queues -> per-half DVE fold
    (output o-permuted CHUNK-MAJOR) + PE transpose + ACT copy -> wmaxT bf16 ->
    per-chunk 32KB DRAM writes -> per-chunk broadcast reads give w_rep
    [p, o', k] (transpose outputs must start at PSUM partition 0, so direct
    placement at partition offsets is impossible; the chunk-major permutation
    lets each wT chunk ship as soon as it exists).
  - DMA scheduling (from packet-level traces): contiguous streams run at
    200-340GB/s per HW queue, but REPLICATED broadcast reads cap at ~130GB/s
    and collapse when several run concurrently, and every hop costs ~2us of
    issue->data latency. So the m broadcast owns sync, w_rep owns scalar, and
    chunk 0 of each is sub-split so the first min gates on partial data.
  - Hot loop per o'-half: bf16 tensor_tensor min (m16rep broadcast over o',
    w_rep broadcast over b'; chunk 0's min runs as a 2x2 (b'-half x o'-half)
    sub-grid so it starts as soon as the first quarter of its inputs lands),
    a halving tensor_tensor max tree over k (256->...->16, all 2x, contiguous
    halves so packing holds), then one 1x tensor_reduce of the last 16
    straight to f32 and a scatter-DMA to the natural [b, o] layout.
"""

import sys

import numpy as np

if "/opt/trn_rl_repo" not in sys.path:
    sys.path.insert(0, "/opt/trn_rl_repo")

B, IN_F, OUT_F, AGG = 1024, 256, 128, 4
N_CORES = 8
B_SH = B // N_CORES  # 128

OG, BG = 8, 16  # partition p = og*BG + bg
BP = B_SH // BG  # 8 batch rows per partition
OPP = OUT_F // OG  # 16 output cols per partition
NT = 2  # o'-halves (chunks of the hot loop)
OC = OPP // NT  # 8 output cols per chunk

_CACHE = {}


EPC = 6  # chunk-1 o'-columns built on-chip via PE selection-matmuls


def _identity_np():
    import ml_dtypes

    return np.eye(128, dtype=ml_dtypes.bfloat16)


def _wmask_np():
    """Selection matrices for the PE-built w_rep columns: wmask[q, j, p] = 1
    iff wmaxT (chunk-major) row q is the one partition p needs for chunk 1's
    column oc = OC - EPC + j, i.e. q = 64 + (p//BG)*OC + (OC - EPC + j)."""
    import ml_dtypes

    q = np.arange(128)[:, None, None]
    p = np.arange(128)[None, None, :]
    j = np.arange(EPC)[None, :, None]
    sel = 64 + (p // BG) * OC + (OC - EPC + j)
    return (q == sel).astype(ml_dtypes.bfloat16)


def emit_core_program(tc, o_d, m_d, w_d, i_d, wm_d):
    """Emit the per-core Tile program.

    o_d: DRAM out [B_SH, OUT_F] f32, m_d: DRAM in [B_SH, IN_F] f32,
    w_d: DRAM in [IN_F, OUT_F*AGG] f32, i_d: DRAM in [128, 128] bf16 identity,
    wm_d: DRAM in [128, EPC, 128] bf16 selection masks (see _wmask_np).
    """
    from contextlib import ExitStack

    import concourse.bass as bass
    from concourse import mybir

    nc = tc.nc
    f32 = mybir.dt.float32
    bf16 = mybir.dt.bfloat16
    AX = mybir.AxisListType
    OP = mybir.AluOpType

    with ExitStack() as ctx:
        const = ctx.enter_context(tc.tile_pool(name="const", bufs=1))
        mintp = ctx.enter_context(tc.tile_pool(name="mintp", bufs=2))
        treep = ctx.enter_context(tc.tile_pool(name="treep", bufs=2))
        ps_tr = ctx.enter_context(tc.tile_pool(name="ps_tr", bufs=2, space="PSUM"))

        # --- queue heads. sync: the m natural load (tiny, unblocks the whole
        # m chain) then its w k-half then ident; scalar: the other w k-half.
        m32n = const.tile([128, IN_F], f32)
        nc.sync.dma_start(out=m32n, in_=m_d)
        w_sb = const.tile([128, 2, OUT_F * AGG], f32)
        wv = w_d.rearrange("(h p) j -> p h j", p=128)
        nc.scalar.dma_start(out=w_sb[:, 0, :], in_=wv[:, 0, :])
        nc.sync.dma_start(out=w_sb[:, 1, :], in_=wv[:, 1, :])
        ident = const.tile([128, 128], bf16)
        nc.sync.dma_start(out=ident, in_=i_d)
        wmask = const.tile([128, EPC, 128], bf16)
        nc.scalar.dma_start(out=wmask, in_=wm_d)

        # m -> bf16 (one tiny, early ACT op) -> DRAM: the broadcast then moves
        # 512KB of already-converted data, and the ACT stream stays clear of
        # the w-chain DMA issues during the hot window.
        m16n = const.tile([128, IN_F], bf16)
        nc.scalar.copy(out=m16n, in_=m32n)
        m16_d = nc.dram_tensor("m16_scratch", [B_SH, IN_F], bf16, kind="Internal").ap()
        nc.sync.dma_start(out=m16_d, in_=m16n)

        # m16rep[p = og*BG+bg, b', k] = m16[bg*BP + b', k], two b'-half chunks
        # solo on the sync queue (chunk 0 unblocks the first min early).
        m16rep = const.tile([128, BP, IN_F], bf16)
        HB = BP * IN_F // 2
        for c in range(2):
            src = bass.AP(
                tensor=m16_d.tensor,
                offset=m16_d.offset + c * HB,
                ap=[[0, OG], [BP * IN_F, BG], [1, HB]],
            )
            half = slice(c * (BP // 2), (c + 1) * (BP // 2))
            nc.sync.dma_start(out=m16rep[:, half, :], in_=src)

        # --- weight fold + transpose, pipelined per k-half h (h=1 lands on
        # the sync queue first). The fold output is o-permuted CHUNK-MAJOR:
        # wmax16[:, h, q] with q = t*64 + og*8 + o'  holds  o = og*16+t*8+o',
        # so wmaxT's partitions [t*64, t*64+64) are exactly chunk t's rows.
        wmax16 = const.tile([128, 2, OUT_F], bf16)
        wmaxT16 = const.tile([128, 2, 128], bf16)
        for h in (1, 0):
            # j = (og*16 + t*8 + o')*4 + a  ==  "(og t o a)" order.
            nc.vector.tensor_reduce(
                out=wmax16[:, h, :].rearrange("p (t og o) -> p t og o", t=NT, og=OG),
                in_=w_sb[:, h, :].rearrange(
                    "p (og t o a) -> p t og o a", og=OG, t=NT, a=AGG
                ),
                axis=AX.X,
                op=OP.max,
            )
            pt = ps_tr.tile([128, 128], bf16, tag="ptr")
            nc.tensor.transpose(pt, wmax16[:, h, :], ident)
            nc.scalar.copy(out=wmaxT16[:, h, :], in_=pt)

        # Per-chunk wT write + replicated read (solo on the scalar queue):
        # w_rep[t][p = og*BG+bg, o', k] = wmaxT row t*64 + og*8 + o'.
        wT_d = nc.dram_tensor("wT_scratch", [OUT_F, IN_F], bf16, kind="Internal").ap()
        wT_v = wT_d.rearrange("(t r) k -> t r k", t=NT)
        # Chunk 1's last EPC columns skip the roundtrip entirely: PE selection
        # matmuls place them in PSUM straight from wmaxT16 (so the DVE can
        # start hot work ~8us before the first w_rep DMA lands).
        wreps = []
        for t in range(NT):
            nc.scalar.dma_start(
                out=wT_v[t], in_=wmaxT16[t * 64 : (t + 1) * 64, :, :]
            )
            noc = OC if t == 0 else OC - EPC
            wr = const.tile([128, noc, IN_F], bf16, name=f"wrep{t}")
            nsub = 2 if t == 0 else 1
            oc_s = noc // nsub
            for s in range(nsub):
                src = bass.AP(
                    tensor=wT_d.tensor,
                    offset=wT_d.offset + (t * 64 + s * oc_s) * IN_F,
                    ap=[[OC * IN_F, OG], [0, BG], [1, oc_s * IN_F]],
                )
                nc.scalar.dma_start(out=wr[:, s * oc_s : (s + 1) * oc_s, :], in_=src)
            wreps.append(wr)

        wmaxT_flat = wmaxT16.rearrange("p h k -> p (h k)")
        we = const.tile([128, EPC, IN_F], bf16, name="we")
        pses = []
        for j in range(EPC):
            pse = ps_tr.tile([128, IN_F], f32, tag=f"pse{j}", bufs=1)
            nc.tensor.matmul(out=pse, lhsT=wmask[:, j, :], rhs=wmaxT_flat)
            pses.append(pse)

        m16v = m16rep.rearrange("p b k -> p b () k")
        part = treep.tile([128, BP, OUT_F // OG], f32, tag="part", bufs=1)

        # --- early hot block on the PE-built columns (o' = 16-EPC .. 16) ----
        for j in range(EPC):
            nc.vector.tensor_copy(we[:, j, :], pses[j])
        mint_e = mintp.tile([128, BP, EPC, IN_F], bf16, tag="mint_e", bufs=1)
        for sb in range(2):  # b'-halves, gated on the two m16rep chunks
            bs = BP // 2
            nc.vector.tensor_tensor(
                out=mint_e[:, sb * bs : (sb + 1) * bs, :, :],
                in0=m16v[:, sb * bs : (sb + 1) * bs, :, :].broadcast_to(
                    (128, bs, EPC, IN_F)
                ),
                in1=we.rearrange("p o k -> p () o k").broadcast_to(
                    (128, bs, EPC, IN_F)
                ),
                op=OP.min,
            )
        cur, n = mint_e, IN_F
        while n > 16:
            h = n // 2
            nxt = treep.tile([128, BP, EPC, h], bf16, tag=f"etree{h}", bufs=1)
            nc.vector.tensor_tensor(
                out=nxt, in0=cur[:, :, :, :h], in1=cur[:, :, :, h:], op=OP.max
            )
            cur, n = nxt, h
        nc.vector.tensor_reduce(
            out=part[:, :, OPP - EPC :], in_=cur, axis=AX.X, op=OP.max
        )
        dst = bass.AP(
            tensor=o_d.tensor,
            offset=o_d.offset + OPP - EPC,
            ap=[[OPP, OG], [BP * OUT_F, BG], [OUT_F, BP], [1, EPC]],
        )
        nc.sync.dma_start(out=dst, in_=part[:, :, OPP - EPC :])

        # --- hot loop: per o'-chunk, bf16 min + halving max-tree over k -----
        for t in range(NT):
            noc = OC if t == 0 else OC - EPC
            mint = mintp.tile([128, BP, OC, IN_F], bf16, tag="mint")
            # Chunk 0's min runs as a (b'-half x o'-half) sub-grid gated on
            # the m/w sub-chunks; chunk 1 is one op.
            grid = (2, 1)[t]
            bs, os_ = BP // grid, noc // grid
            for sb in range(grid):
                for so in range(grid):
                    nc.vector.tensor_tensor(
                        out=mint[
                            :, sb * bs : (sb + 1) * bs, so * os_ : (so + 1) * os_, :
                        ],
                        in0=m16v[:, sb * bs : (sb + 1) * bs, :, :].broadcast_to(
                            (128, bs, os_, IN_F)
                        ),
                        in1=wreps[t][:, so * os_ : (so + 1) * os_, :]
                        .rearrange("p o k -> p () o k")
                        .broadcast_to((128, bs, os_, IN_F)),
                        op=OP.min,
                    )
            cur, n = mint[:, :, :noc, :], IN_F
            while n > 16:
                h = n // 2
                nxt = treep.tile([128, BP, OC, h], bf16, tag=f"tree{h}")
                nc.vector.tensor_tensor(
                    out=nxt[:, :, :noc, :],
                    in0=cur[:, :, :, :h],
                    in1=cur[:, :, :, h:],
                    op=OP.max,
                )
                cur, n = nxt[:, :, :noc, :], h
            nc.vector.tensor_reduce(
                out=part[:, :, t * OC : t * OC + noc], in_=cur, axis=AX.X, op=OP.max
            )
            dst = bass.AP(
                tensor=o_d.tensor,
                offset=o_d.offset + t * OC,
                ap=[[OPP, OG], [BP * OUT_F, BG], [OUT_F, BP], [1, noc]],
            )
            nc.sync.dma_start(out=dst, in_=part[:, :, t * OC : t * OC + noc])


def _build():
    if "nc" in _CACHE:
        return _CACHE["nc"]
    import concourse.bacc as bacc
    import concourse.tile as tile
    from concourse import mybir

    f32 = mybir.dt.float32
    bf16 = mybir.dt.bfloat16
    nc = bacc.Bacc(
        "TRN2",
        target_bir_lowering=False,
        debug=False,
        enable_asserts=True,
        num_devices=N_CORES,
    )
    m_d = nc.dram_tensor("m0", [B_SH, IN_F], f32, kind="ExternalInput").ap()
    w_d = nc.dram_tensor("w0", [IN_F, OUT_F * AGG], f32, kind="ExternalInput").ap()
    i_d = nc.dram_tensor("ident0", [128, 128], bf16, kind="ExternalInput").ap()
    wm_d = nc.dram_tensor("wmask0", [128, EPC, 128], bf16, kind="ExternalInput").ap()
    o_d = nc.dram_tensor("out0", [B_SH, OUT_F], f32, kind="ExternalOutput").ap()
    with tile.TileContext(nc) as tc:
        emit_core_program(tc, o_d, m_d, w_d, i_d, wm_d)
    nc.compile()
    _CACHE["nc"] = nc
    return nc


def run(m, weight, trace=False, **spmd_kwargs):
    """Run on 8 NeuronCores; returns (full_output, BassKernelResults)."""
    from concourse.bass_utils import run_bass_kernel_spmd

    nc = _build()
    m = np.ascontiguousarray(np.asarray(m, dtype=np.float32))
    weight = np.ascontiguousarray(np.asarray(weight, dtype=np.float32))
    assert m.shape == (B, IN_F) and weight.shape == (IN_F, OUT_F * AGG)
    ident = _identity_np()
    wmask = _wmask_np()
    in_maps = [
        {
            "m0": m[i * B_SH : (i + 1) * B_SH],
            "w0": weight,
            "ident0": ident,
            "wmask0": wmask,
        }
        for i in range(N_CORES)
    ]
    res = run_bass_kernel_spmd(
        nc, in_maps, core_ids=list(range(N_CORES)), trace=trace, **spmd_kwargs
    )
    out = np.concatenate([res.results[i]["out0"] for i in range(N_CORES)], axis=0)
    return out, res


def kernel(m, weight, agg_features=AGG, **_ignored):
    assert int(agg_features) == AGG
    out, _ = run(m, weight, trace=False)
    return out.astype(np.float32)


# revision 39
# speedup vs baseline: 1.1733x; 1.0177x over previous
"""Trainium2 Bass kernel for nn_MaxMinAgg.

Computes, for full inputs m [1024, 256] f32 and weight [256, 512] f32:
    z[b, j]  = max_k min(m[b, k], weight[k, j])          (tropical max-min matmul)
    out[b,o] = max_a z[b, 4*o + a]                       (max-pool over AGG=4 groups)

Identity: max_a min(x, w_a) = min(x, max_a w_a): the AGG max-pool folds into the
weight (wmax[k, o] = max_a weight[k, 4o+a]), so
    out[b, o] = max_k min(m[b, k], wmax[k, o])

Distribution: data-parallel over batch across 8 NeuronCores (128 rows each);
weight replicated.

Per-core algorithm (final — bf16, low-replication layout, max-tree, short DMA
chains, PE-built early weight columns):
  - Chunk 1's last EPC=6 o'-columns of w_rep are built ON-CHIP: host-fed 0/1
    selection matrices (wmask0) are matmul'd against wmaxT on the idle PE and
    CAST-copied PSUM->SBUF by the DVE, so the DVE starts hot min/tree work
    ~8us before the first replicated w_rep DMA lands (fills the startup idle
    window that the DRAM-roundtrip latency otherwise leaves).
  - All compute in bf16: min/max are exact selections, so the only error is the
    initial f32->bf16 rounding (<= 2^-9 relative, far under the 2e-2 gate), and
    bf16 unlocks the DVE's 2x packed mode for tensor_tensor (tensor_reduce is
    always 1x, hence the halving max-tree below).
  - Partition layout p = og*16 + bg (OG=8 output groups x BG=16 batch groups).
    Partition p owns the full k-reduction for an 8b' x 16o' output tile, so
    k (=256) lives on the free axis and there is NO cross-partition combine.
  - m path: ONE replicated broadcast read straight from the input (1MB f32,
    8KB-contiguous runs, two b'-half chunks solo on the sync queue), then two
    ACT converts produce m16rep [p, b', k] bf16 — no DRAM scratch hop.
  - w path: 512KB load split across both HW queues -> per-half DVE fold
    (output o-permuted CHUNK-MAJOR) + PE transpose + ACT copy -> wmaxT bf16 ->
    per-chunk 32KB DRAM writes -> per-chunk broadcast reads give w_rep
    [p, o', k] (transpose outputs must start at PSUM partition 0, so direct
    placement at partition offsets is impossible; the chunk-major permutation
    lets each wT chunk ship as soon as it exists).
  - DMA scheduling (from packet-level traces): contiguous streams run at
    200-340GB/s per HW queue, but REPLICATED broadcast reads cap at ~130GB/s
    and collapse when several run concurrently, and every hop costs ~2us of
    issue->data latency. So the m broadcast owns sync, w_rep owns scalar, and
    chunk 0 of each is sub-split so the first min gates on partial data.
  - Hot loop per o'-half: bf16 tensor_tensor min (m16rep broadcast over o',
    w_rep broadcast over b'; chunk 0's min runs as a 2x2 (b'-half x o'-half)
    sub-grid so it starts as soon as the first quarter of its inputs lands),
    a halving tensor_tensor max tree over k (256->...->16, all 2x, contiguous
    halves so packing holds), then one 1x tensor_reduce of the last 16
    straight to f32 and a scatter-DMA to the natural [b, o] layout.
"""

import sys

import numpy as np

if "/opt/trn_rl_repo" not in sys.path:
    sys.path.insert(0, "/opt/trn_rl_repo")

B, IN_F, OUT_F, AGG = 1024, 256, 128, 4
N_CORES = 8
B_SH = B // N_CORES  # 128

OG, BG = 8, 16  # partition p = og*BG + bg
BP = B_SH // BG  # 8 batch rows per partition
OPP = OUT_F // OG  # 16 output cols per partition
NT = 2  # o'-halves (chunks of the hot loop)
OC = OPP // NT  # 8 output cols per chunk

_CACHE = {}


EPC = 6  # chunk-1 o'-columns built on-chip via PE selection-matmuls


def _identity_np():
    import ml_dtypes

    return np.eye(128, dtype=ml_dtypes.bfloat16)


def _wmask_np():
    """Selection matrices for the PE-built w_rep columns: wmask[q, j, p] = 1
    iff wmaxT (chunk-major) row q is the one partition p needs for chunk 1's
    column oc = OC - EPC + j, i.e. q = 64 + (p//BG)*OC + (OC - EPC + j)."""
    import ml_dtypes

    q = np.arange(128)[:, None, None]
    p = np.arange(128)[None, None, :]
    j = np.arange(EPC)[None, :, None]
    sel = 64 + (p // BG) * OC + (OC - EPC + j)
    return (q == sel).astype(ml_dtypes.bfloat16)


def emit_core_program(tc, o_d, m_d, w_d, i_d, wm_d):
    """Emit the per-core Tile program.

    o_d: DRAM out [B_SH, OUT_F] f32, m_d: DRAM in [B_SH, IN_F] f32,
    w_d: DRAM in [IN_F, OUT_F*AGG] f32, i_d: DRAM in [128, 128] bf16 identity,
    wm_d: DRAM in [128, EPC, 128] bf16 selection masks (see _wmask_np).
    """
    from contextlib import ExitStack

    import concourse.bass as bass
    from concourse import mybir

    nc = tc.nc
    f32 = mybir.dt.float32
    bf16 = mybir.dt.bfloat16
    AX = mybir.AxisListType
    OP = mybir.AluOpType

    with ExitStack() as ctx:
        const = ctx.enter_context(tc.tile_pool(name="const", bufs=1))
        mintp = ctx.enter_context(tc.tile_pool(name="mintp", bufs=2))
        treep = ctx.enter_context(tc.tile_pool(name="treep", bufs=2))
        ps_tr = ctx.enter_context(tc.tile_pool(name="ps_tr", bufs=2, space="PSUM"))

        # --- queue heads. sync: the m natural load (tiny, unblocks the whole
        # m chain) then its w k-half then ident; scalar: the other w k-half.
        m32n = const.tile([128, IN_F], f32)
        nc.sync.dma_start(out=m32n, in_=m_d)
        w_sb = const.tile([128, 2, OUT_F * AGG], f32)
        wv = w_d.rearrange("(h p) j -> p h j", p=128)
        nc.scalar.dma_start(out=w_sb[:, 0, :], in_=wv[:, 0, :])
        nc.sync.dma_start(out=w_sb[:, 1, :], in_=wv[:, 1, :])
        ident = const.tile([128, 128], bf16)
        nc.sync.dma_start(out=ident, in_=i_d)
        wmask = const.tile([128, EPC, 128], bf16)
        nc.scalar.dma_start(out=wmask, in_=wm_d)

        # m -> bf16 (one tiny, early ACT op) -> DRAM: the broadcast then moves
        # 512KB of already-converted data, and the ACT stream stays clear of
        # the w-chain DMA issues during the hot window.
        m16n = const.tile([128, IN_F], bf16)
        nc.scalar.copy(out=m16n, in_=m32n)
        m16_d = nc.dram_tensor("m16_scratch", [B_SH, IN_F], bf16, kind="Internal").ap()
        nc.sync.dma_start(out=m16_d, in_=m16n)

        # m16rep[p = og*BG+bg, b', k] = m16[bg*BP + b', k], two b'-half chunks
        # solo on the sync queue (chunk 0 unblocks the first min early).
        m16rep = const.tile([128, BP, IN_F], bf16)
        HB = BP * IN_F // 2
        for c in range(2):
            src = bass.AP(
                tensor=m16_d.tensor,
                offset=m16_d.offset + c * HB,
                ap=[[0, OG], [BP * IN_F, BG], [1, HB]],
            )
            half = slice(c * (BP // 2), (c + 1) * (BP // 2))
            nc.sync.dma_start(out=m16rep[:, half, :], in_=src)

        # --- weight fold + transpose, pipelined per k-half h (h=1 lands on
        # the sync queue first). The fold output is o-permuted CHUNK-MAJOR:
        # wmax16[:, h, q] with q = t*64 + og*8 + o'  holds  o = og*16+t*8+o',
        # so wmaxT's partitions [t*64, t*64+64) are exactly chunk t's rows.
        wmax16 = const.tile([128, 2, OUT_F], bf16)
        wmaxT16 = const.tile([128, 2, 128], bf16)
        for h in (1, 0):
            # j = (og*16 + t*8 + o')*4 + a  ==  "(og t o a)" order.
            nc.vector.tensor_reduce(
                out=wmax16[:, h, :].rearrange("p (t og o) -> p t og o", t=NT, og=OG),
                in_=w_sb[:, h, :].rearrange(
                    "p (og t o a) -> p t og o a", og=OG, t=NT, a=AGG
                ),
                axis=AX.X,
                op=OP.max,
            )
            pt = ps_tr.tile([128, 128], bf16, tag="ptr")
            nc.tensor.transpose(pt, wmax16[:, h, :], ident)
            nc.scalar.copy(out=wmaxT16[:, h, :], in_=pt)

        # Per-chunk wT write + replicated read (solo on the scalar queue):
        # w_rep[t][p = og*BG+bg, o', k] = wmaxT row t*64 + og*8 + o'.
        wT_d = nc.dram_tensor("wT_scratch", [OUT_F, IN_F], bf16, kind="Internal").ap()
        wT_v = wT_d.rearrange("(t r) k -> t r k", t=NT)
        # Chunk 1's last EPC columns skip the roundtrip entirely: PE selection
        # matmuls place them in PSUM straight from wmaxT16 (so the DVE can
        # start hot work ~8us before the first w_rep DMA lands).
        wreps = []
        for t in range(NT):
            nc.scalar.dma_start(
                out=wT_v[t], in_=wmaxT16[t * 64 : (t + 1) * 64, :, :]
            )
            noc = OC if t == 0 else OC - EPC
            wr = const.tile([128, noc, IN_F], bf16, name=f"wrep{t}")
            nsub = 2 if t == 0 else 1
            oc_s = noc // nsub
            for s in range(nsub):
                src = bass.AP(
                    tensor=wT_d.tensor,
                    offset=wT_d.offset + (t * 64 + s * oc_s) * IN_F,
                    ap=[[OC * IN_F, OG], [0, BG], [1, oc_s * IN_F]],
                )
                nc.scalar.dma_start(out=wr[:, s * oc_s : (s + 1) * oc_s, :], in_=src)
            wreps.append(wr)

        wmaxT_flat = wmaxT16.rearrange("p h k -> p (h k)")
        we = const.tile([128, EPC, IN_F], bf16, name="we")
        pses = []
        for j in range(EPC):
            pse = ps_tr.tile([128, IN_F], f32, tag=f"pse{j}", bufs=1)
            nc.tensor.matmul(out=pse, lhsT=wmask[:, j, :], rhs=wmaxT_flat)
            pses.append(pse)

        m16v = m16rep.rearrange("p b k -> p b () k")
        part = treep.tile([128, BP, OUT_F // OG], f32, tag="part", bufs=1)

        # --- early hot block on the PE-built columns (o' = 16-EPC .. 16) ----
        for j in range(EPC):
            nc.vector.tensor_copy(we[:, j, :], pses[j])
        mint_e = mintp.tile([128, BP, EPC, IN_F], bf16, tag="mint_e", bufs=1)
        for sb in range(2):  # b'-halves, gated on the two m16rep chunks
            bs = BP // 2
            nc.vector.tensor_tensor(
                out=mint_e[:, sb * bs : (sb + 1) * bs, :, :],
                in0=m16v[:, sb * bs : (sb + 1) * bs, :, :].broadcast_to(
                    (128, bs, EPC, IN_F)
                ),
                in1=we.rearrange("p o k -> p () o k").broadcast_to(
                    (128, bs, EPC, IN_F)
                ),
                op=OP.min,
            )
        cur, n = mint_e, IN_F
        while n > 16:
            h = n // 2
            nxt = treep.tile([128, BP, EPC, h], bf16, tag=f"etree{h}", bufs=1)
            nc.vector.tensor_tensor(
                out=nxt, in0=cur[:, :, :, :h], in1=cur[:, :, :, h:], op=OP.max
            )
            cur, n = nxt, h
        nc.vector.tensor_reduce(
            out=part[:, :, OPP - EPC :], in_=cur, axis=AX.X, op=OP.max
        )

        # --- hot loop: per o'-chunk, bf16 min + halving max-tree over k -----
        for t in range(NT):
            noc = OC if t == 0 else OC - EPC
            mint = mintp.tile([128, BP, OC, IN_F], bf16, tag="mint")
            # Chunk 0's min runs as a (b'-half x o'-half) sub-grid gated on
            # the m/w sub-chunks; chunk 1 is one op.
            grid = (2, 1)[t]
            bs, os_ = BP // grid, noc // grid
            for sb in range(grid):
                for so in range(grid):
                    nc.vector.tensor_tensor(
                        out=mint[
                            :, sb * bs : (sb + 1) * bs, so * os_ : (so + 1) * os_, :
                        ],
                        in0=m16v[:, sb * bs : (sb + 1) * bs, :, :].broadcast_to(
                            (128, bs, os_, IN_F)
                        ),
                        in1=wreps[t][:, so * os_ : (so + 1) * os_, :]
                        .rearrange("p o k -> p () o k")
                        .broadcast_to((128, bs, os_, IN_F)),
                        op=OP.min,
                    )
            cur, n = mint[:, :, :noc, :], IN_F
            while n > 16:
                h = n // 2
                nxt = treep.tile([128, BP, OC, h], bf16, tag=f"tree{h}")
                nc.vector.tensor_tensor(
                    out=nxt[:, :, :noc, :],
                    in0=cur[:, :, :, :h],
                    in1=cur[:, :, :, h:],
                    op=OP.max,
                )
                cur, n = nxt[:, :, :noc, :], h
            nc.vector.tensor_reduce(
                out=part[:, :, t * OC : t * OC + noc], in_=cur, axis=AX.X, op=OP.max
            )

        # Both chunks' o' are adjacent in `part`, so one scatter with 64B
        # contiguous runs writes the natural [b, o] layout.
        dst = bass.AP(
            tensor=o_d.tensor,
            offset=o_d.offset,
            ap=[[OPP, OG], [BP * OUT_F, BG], [OUT_F, BP], [1, OPP]],
        )
        nc.sync.dma_start(out=dst, in_=part)


def _build():
    if "nc" in _CACHE:
        return _CACHE["nc"]
    import concourse.bacc as bacc
    import concourse.tile as tile
    from concourse import mybir

    f32 = mybir.dt.float32
    bf16 = mybir.dt.bfloat16
    nc = bacc.Bacc(
        "TRN2",
        target_bir_lowering=False,
        debug=False,
        enable_asserts=True,
        num_devices=N_CORES,
    )
    m_d = nc.dram_tensor("m0", [B_SH, IN_F], f32, kind="ExternalInput").ap()
    w_d = nc.dram_tensor("w0", [IN_F, OUT_F * AGG], f32, kind="ExternalInput").ap()
    i_d = nc.dram_tensor("ident0", [128, 128], bf16, kind="ExternalInput").ap()
    wm_d = nc.dram_tensor("wmask0", [128, EPC, 128], bf16, kind="ExternalInput").ap()
    o_d = nc.dram_tensor("out0", [B_SH, OUT_F], f32, kind="ExternalOutput").ap()
    with tile.TileContext(nc) as tc:
        emit_core_program(tc, o_d, m_d, w_d, i_d, wm_d)
    nc.compile()
    _CACHE["nc"] = nc
    return nc


def run(m, weight, trace=False, **spmd_kwargs):
    """Run on 8 NeuronCores; returns (full_output, BassKernelResults)."""
    from concourse.bass_utils import run_bass_kernel_spmd

    nc = _build()
    m = np.ascontiguousarray(np.asarray(m, dtype=np.float32))
    weight = np.ascontiguousarray(np.asarray(weight, dtype=np.float32))
    assert m.shape == (B, IN_F) and weight.shape == (IN_F, OUT_F * AGG)
    ident = _identity_np()
    wmask = _wmask_np()
    in_maps = [
        {
            "m0": m[i * B_SH : (i + 1) * B_SH],
            "w0": weight,
            "ident0": ident,
            "wmask0": wmask,
        }
        for i in range(N_CORES)
    ]
    res = run_bass_kernel_spmd(
        nc, in_maps, core_ids=list(range(N_CORES)), trace=trace, **spmd_kwargs
    )
    out = np.concatenate([res.results[i]["out0"] for i in range(N_CORES)], axis=0)
    return out, res


def kernel(m, weight, agg_features=AGG, **_ignored):
    assert int(agg_features) == AGG
    out, _ = run(m, weight, trace=False)
    return out.astype(np.float32)
